# revision 1
# baseline (speedup 1.0000x reference)
"""Causal self-attention (QKV proj + RoPE + causal SDPA + out proj) on 8 trn2 cores.

Sharding: tensor-parallel over heads. Each core owns 2 of 16 heads:
  - Wqkv column-split (the core's q/k/v head rows), Wproj row-split.
  - Each core computes a full-shape partial of the output projection;
    the 8 partials are summed (and transposed back) on the host.

Device-side layout trick: everything runs transposed. The host feeds
x^T [C, B*T]; the QKV projection computes qkv^T = Wslice @ x with the
head dim on partitions, which is exactly what Q@K^T and the output
projection want as inputs, so no on-chip transposes are needed except
V (done with DMA xbar transposes, off the critical engines).
"""
import sys

sys.path.insert(0, "/opt/trn_rl_repo")

import numpy as np
import ml_dtypes

import concourse.bacc as bacc
import concourse.mybir as mybir
import concourse.tile as tile
from concourse.bass_utils import run_bass_kernel_spmd

N_CORES = 8
C = 2048
H = 16
D = 128
HPC = H // N_CORES          # heads per core = 2
PB = 512                    # row panel width
JB = 128                    # key tile width
NEG = -1.0e30
ROPE_BASE = 10000.0

BF = mybir.dt.bfloat16
F32 = mybir.dt.float32


def build_module(B, T):
    BT = B * T
    CC = C // 128            # contraction chunks for the projection
    FT = 3 * HPC             # qkv f-tiles per core (q0 q1 k0 k1 v0 v1)
    NPB = T // PB            # panels per batch
    NOC = C // 128           # out-proj column tiles
    scale = 1.0 / float(np.sqrt(D))

    nc = bacc.Bacc("TRN2", target_bir_lowering=False, debug=False,
                   num_devices=N_CORES)

    # x pre-tiled on host: xtiles[g, p, cc*PB + r] = x[g*PB + r, cc*128 + p]
    # -> one DMA per panel with 16KB contiguous runs (descriptor-rate bound
    #    HWDGE moves ~74GB/s at 1KB runs; long runs unlock full bandwidth)
    xtiles = nc.dram_tensor("xtiles", [BT // PB, 128, CC * PB], BF,
                            kind="ExternalInput").ap()
    wqkvT = nc.dram_tensor("wqkvT", [C, FT * 128], BF, kind="ExternalInput").ap()
    wprojT = nc.dram_tensor("wprojT", [HPC * 128, C], BF, kind="ExternalInput").ap()
    cosT = nc.dram_tensor("cosT", [128, T], BF, kind="ExternalInput").ap()
    sinT = nc.dram_tensor("sinT", [128, T], F32, kind="ExternalInput").ap()
    maskT = nc.dram_tensor("maskT", [128, 896], F32, kind="ExternalInput").ap()
    permT = nc.dram_tensor("permT", [128, 128], BF, kind="ExternalInput").ap()
    zout = nc.dram_tensor("zout", [C, BT], BF, kind="ExternalOutput").ap()

    with tile.TileContext(nc) as tc:
        with tc.tile_pool(name="sb", bufs=1) as sb, \
             tc.tile_pool(name="ps", bufs=1, space="PSUM") as ps:
            # ---- resident constants ----
            wqkv_sb = sb.tile([128, CC, FT * 128], BF, tag="wqkv", bufs=1)
            nc.sync.dma_start(
                out=wqkv_sb[:],
                in_=wqkvT.rearrange("(cc p) f -> p cc f", p=128))
            wproj_sb = sb.tile([128, HPC, C], BF, tag="wproj", bufs=1)
            nc.sync.dma_start(
                out=wproj_sb[:],
                in_=wprojT.rearrange("(hh p) o -> p hh o", p=128))
            cos_sb = sb.tile([128, T], BF, tag="cos", bufs=1)
            nc.sync.dma_start(out=cos_sb[:], in_=cosT)
            sin_sb = sb.tile([128, T], F32, tag="sin", bufs=1)
            nc.sync.dma_start(out=sin_sb[:], in_=sinT)
            mask_sb = sb.tile([128, 896], F32, tag="mask", bufs=1)
            nc.sync.dma_start(out=mask_sb[:], in_=maskT)
            perm_sb = sb.tile([128, 128], BF, tag="perm", bufs=1)
            nc.sync.dma_start(out=perm_sb[:], in_=permT)
            ones_col = sb.tile([128, 1], BF, tag="ones_c", bufs=1)
            nc.vector.memset(ones_col[:], 1.0)
            ones_row = sb.tile([1, 128], BF, tag="ones_r", bufs=1)
            nc.vector.memset(ones_row[:], 1.0)


            def emit_outproj(ypair, b, pp):
                r0g = b * T + pp * PB
                for oc in range(NOC):
                    zps = ps.tile([128, PB], F32, tag="mm", bufs=4)
                    for hh in range(HPC):
                        nc.tensor.matmul(
                            zps[:],
                            lhsT=wproj_sb[:, hh, oc * 128:(oc + 1) * 128],
                            rhs=ypair[hh][:],
                            start=(hh == 0), stop=(hh == HPC - 1))
                    zst = sb.tile([128, PB], BF, tag="zst", bufs=4)
                    nc.vector.tensor_copy(out=zst[:], in_=zps[:])
                    nc.gpsimd.dma_start(
                        out=zout[oc * 128:(oc + 1) * 128, r0g:r0g + PB],
                        in_=zst[:])

            HC = CC // 2

            def load_xt(b, pp):
                g = b * NPB + pp
                xta = sb.tile([128, HC, PB], BF, tag="xta", bufs=3,
                              name=f"xta_{b}_{pp}")
                xtb = sb.tile([128, HC, PB], BF, tag="xtb", bufs=3,
                              name=f"xtb_{b}_{pp}")
                src = xtiles[g].rearrange("p (cc r) -> p cc r", r=PB)
                nc.sync.dma_start(out=xta[:], in_=src[:, :HC, :])
                nc.gpsimd.dma_start(out=xtb[:], in_=src[:, HC:, :])
                return (xta, xtb)

            pending = None
            panels = [(b, pp) for b in range(B) for pp in range(NPB)]
            xt_q = [load_xt(*panels[0])]
            if len(panels) > 1:
                xt_q.append(load_xt(*panels[1]))
            gidx = 0
            for b in range(B):
                # ---------- projection + rope for batch b ----------
                q_t = [sb.tile([128, T], BF, tag=f"q{h}", bufs=2,
                               name=f"q{h}_{b}")
                       for h in range(HPC)]
                k_t = [sb.tile([128, T], BF, tag=f"k{h}", bufs=2,
                               name=f"k{h}_{b}")
                       for h in range(HPC)]
                v_t = [sb.tile([128, T // 128, 128], BF, tag=f"v{h}", bufs=2,
                               name=f"v{h}_{b}")
                       for h in range(HPC)]
                for pp in range(NPB):
                    r0g = b * T + pp * PB
                    ts = slice(pp * PB, pp * PB + PB)
                    xt = xt_q.pop(0)
                    if gidx + 2 < len(panels):
                        xt_q.append(load_xt(*panels[gidx + 2]))
                    gidx += 1
                    for ft in range(FT):
                        pps = ps.tile([128, PB], F32, tag="mm", bufs=4)
                        for cc in range(CC):
                            xsrc = xt[0][:, cc, :] if cc < HC \
                                else xt[1][:, cc - HC, :]
                            nc.tensor.matmul(
                                pps[:],
                                lhsT=wqkv_sb[:, cc, ft * 128:(ft + 1) * 128],
                                rhs=xsrc,
                                start=(cc == 0), stop=(cc == CC - 1))
                        if ft < 2 * HPC:   # q or k: apply rope
                            raw = sb.tile([128, PB], BF, tag="qkraw", bufs=2)
                            nc.scalar.copy(out=raw[:], in_=pps[:])
                            rot = ps.tile([128, PB], F32, tag="mm", bufs=4)
                            nc.tensor.matmul(rot[:], lhsT=perm_sb[:],
                                             rhs=raw[:], start=True, stop=True)
                            t1 = sb.tile([128, PB], F32, tag="t1", bufs=2)
                            nc.vector.tensor_mul(out=t1[:], in0=raw[:],
                                                 in1=cos_sb[:, ts])
                            t2 = sb.tile([128, PB], F32, tag="t2", bufs=2)
                            nc.vector.tensor_mul(out=t2[:], in0=rot[:],
                                                 in1=sin_sb[:, ts])
                            dest = (q_t if ft < HPC else k_t)[ft % HPC]
                            nc.vector.tensor_add(out=dest[:, ts], in0=t1[:],
                                                 in1=t2[:])
                        else:              # v: stage + dma-transpose
                            h = ft - 2 * HPC
                            vst = sb.tile([128, PB], BF, tag="vstage", bufs=2)
                            nc.scalar.copy(out=vst[:], in_=pps[:])
                            teng = nc.scalar
                            for q4 in range(PB // 128):
                                jt = pp * (PB // 128) + q4
                                teng.dma_start_transpose(
                                    out=v_t[h][:, jt, :],
                                    in_=vst[:, q4 * 128:(q4 + 1) * 128])
                    if pp == 0 and pending is not None:
                        emit_outproj(*pending)
                        pending = None
                # ---------- attention + out-proj for batch b ----------
                for pp in range(NPB):
                    nj = (pp + 1) * (PB // JB)
                    q0 = pp * PB
                    ytil = [ps.tile([128, PB], F32, tag="ytil", bufs=2,
                                    name=f"ytil{h}_{b}_{pp}")
                            for h in range(HPC)]
                    denom = [ps.tile([1, PB], F32, tag="small", bufs=2,
                                     name=f"den{h}_{b}_{pp}")
                             for h in range(HPC)]

                    def emit_S(h, j):
                        kk = j - pp * (PB // JB)
                        lo = max(kk, 0) * 128   # columns < lo fully masked
                        sps = ps.tile([128, PB], F32, tag="mm", bufs=4,
                                      name=f"s{h}_{b}_{pp}_{j}")
                        nc.tensor.matmul(
                            sps[:, lo:PB],
                            lhsT=k_t[h][:, j * JB:(j + 1) * JB],
                            rhs=q_t[h][:, q0 + lo:q0 + PB],
                            start=True, stop=True)
                        return sps

                    def emit_rest(h, j, sps):
                        kk = j - pp * (PB // JB)
                        lo = max(kk, 0) * 128
                        e = sb.tile([128, PB], BF, tag="e", bufs=4,
                                    name=f"e{h}_{b}_{pp}_{j}")
                        if kk >= 0:
                            # triangular 128-col slice gets the mask; the
                            # rest of the block is fully valid
                            nc.vector.scalar_tensor_tensor(
                                out=sps[:, lo:lo + 128],
                                in0=sps[:, lo:lo + 128], scalar=scale,
                                in1=mask_sb[:, 384:512],
                                op0=mybir.AluOpType.mult,
                                op1=mybir.AluOpType.add)
                            nc.scalar.activation(
                                out=e[:, lo:lo + 128], in_=sps[:, lo:lo + 128],
                                func=mybir.ActivationFunctionType.Exp)
                            if lo + 128 < PB:
                                nc.scalar.activation(
                                    out=e[:, lo + 128:PB],
                                    in_=sps[:, lo + 128:PB],
                                    func=mybir.ActivationFunctionType.Exp,
                                    scale=scale)
                        else:
                            nc.scalar.activation(
                                out=e[:, lo:PB], in_=sps[:, lo:PB],
                                func=mybir.ActivationFunctionType.Exp,
                                scale=scale)
                        nc.tensor.matmul(denom[h][:, lo:PB], lhsT=ones_col[:],
                                         rhs=e[:, lo:PB], start=(j == 0),
                                         stop=(j == nj - 1))
                        nc.tensor.matmul(ytil[h][:, lo:PB],
                                         lhsT=v_t[h][:, j, :],
                                         rhs=e[:, lo:PB], start=(j == 0),
                                         stop=(j == nj - 1))

                    jobs = [(h, j) for j in range(nj) for h in range(HPC)]
                    spss = {jobs[0]: emit_S(*jobs[0]),
                            jobs[1]: emit_S(*jobs[1])}
                    for idx, (h, j) in enumerate(jobs):
                        if idx + 2 < len(jobs):
                            spss[jobs[idx + 2]] = emit_S(*jobs[idx + 2])
                        emit_rest(h, j, spss.pop((h, j)))

                    ypair = []
                    for h in range(HPC):
                        dbf = sb.tile([1, PB], BF, tag="dbf", bufs=2)
                        nc.scalar.copy(out=dbf[:], in_=denom[h][:])
                        bc = ps.tile([128, PB], F32, tag="small", bufs=2,
                                     name=f"bc{h}_{b}_{pp}")
                        nc.tensor.matmul(bc[:], lhsT=ones_row[:],
                                         rhs=dbf[:], start=True, stop=True)
                        rec = sb.tile([128, PB], F32, tag="rec", bufs=2)
                        nc.vector.reciprocal_approx_fast(out=rec[:], in_=bc[:])
                        yp = sb.tile([128, PB], BF, tag="yp", bufs=6)
                        nc.vector.tensor_mul(out=yp[:], in0=ytil[h][:],
                                             in1=rec[:])
                        ypair.append(yp)
                    if pending is not None:
                        emit_outproj(*pending)
                    pending = (ypair, b, pp)
            emit_outproj(*pending)

    nc.compile()
    return nc


_module_cache = {}


def _get_module(B, T):
    key = (B, T)
    if key not in _module_cache:
        _module_cache[key] = build_module(B, T)
    return _module_cache[key]


def _host_prep(x, Wqkv, Wproj, B, T):
    bf16 = ml_dtypes.bfloat16
    BT = B * T
    NP = BT // PB
    CC = C // 128
    x2 = x.reshape(NP, PB, CC, 128)
    xtiles = np.ascontiguousarray(
        x2.transpose(0, 3, 2, 1).reshape(NP, 128, CC * PB)).astype(bf16)

    inv = 1.0 / (ROPE_BASE ** (np.arange(0, D, 2, dtype=np.float32) / D))
    t = np.arange(T, dtype=np.float32)
    fr = np.outer(t, inv)                      # [T, 64]
    emb = np.concatenate([fr, fr], -1)         # [T, 128]
    cosT = np.ascontiguousarray(np.cos(emb).T).astype(bf16)
    sinT = np.ascontiguousarray(np.sin(emb).T).astype(np.float32)

    g = np.arange(896)[None, :]
    p = np.arange(128)[:, None]
    maskT = np.where(g >= p + 384, 0.0, NEG).astype(np.float32)

    permT = np.zeros((128, 128), np.float32)
    for j in range(64):
        permT[j, j + 64] = 1.0                 # rot[i] = q[i-64] for i>=64
    for j in range(64, 128):
        permT[j, j - 64] = -1.0                # rot[i] = -q[i+64] for i<64
    permT = permT.astype(bf16)

    in_maps = []
    for c in range(N_CORES):
        heads = [HPC * c + h for h in range(HPC)]
        rows = []
        for blk in range(3):                   # q, k, v blocks of Wqkv
            for h in heads:
                r0 = blk * C + h * D
                rows.append(Wqkv[r0:r0 + D])
        wslice = np.concatenate(rows, 0)       # [FT*128, C]
        wqkvT = np.ascontiguousarray(wslice.T).astype(bf16)
        cols = np.concatenate([np.arange(h * D, (h + 1) * D) for h in heads])
        wprojT = np.ascontiguousarray(Wproj[:, cols].T).astype(bf16)
        in_maps.append({
            "xtiles": xtiles,
            "wqkvT": wqkvT,
            "wprojT": wprojT,
            "cosT": cosT,
            "sinT": sinT,
            "maskT": maskT,
            "permT": permT,
        })
    return in_maps


last_results = None


def kernel(x, Wqkv, Wproj, _trace=False, _trace_kwargs=None):
    global last_results
    x = np.asarray(x, dtype=np.float32)
    Wqkv = np.asarray(Wqkv, dtype=np.float32)
    Wproj = np.asarray(Wproj, dtype=np.float32)
    B, T, _C = x.shape
    assert _C == C and T % PB == 0

    nc = _get_module(B, T)
    in_maps = _host_prep(x, Wqkv, Wproj, B, T)
    res = run_bass_kernel_spmd(nc, in_maps, core_ids=list(range(N_CORES)),
                               trace=_trace, **(_trace_kwargs or {}))
    last_results = res
    z = res.results[0]["zout"].astype(np.float32)
    for c in range(1, N_CORES):
        z += res.results[c]["zout"].astype(np.float32)
    y = np.ascontiguousarray(z.T).reshape(B, T, C)
    return y



# revision 8
# speedup vs baseline: 1.4038x; 1.4038x over previous
"""Causal self-attention (QKV proj + RoPE + causal SDPA + out proj) on 8 trn2 cores.

Sharding: tensor-parallel over heads. Each core owns 2 of 16 heads:
  - Wqkv column-split (the core's q/k/v head rows), Wproj row-split.
  - Each core computes a full-shape partial of the output projection;
    the 8 partials are summed (and transposed back) on the host.

Device-side layout: everything runs transposed (x^T on chip); the QKV
projection computes qkv^T = Wslice @ x with the head dim on partitions,
which is what Q@K^T and the output projection want as inputs. V is
transposed on chip with DMA xbar transposes.

Pipelined schedule: the per-panel work is split into three stages
(QKV-proj+rope for panel g, attention for panel g-1, out-proj for panel
g-2) whose instruction chunks are interleaved in emission order, so every
engine consumes operands produced at least a stage earlier and the tensor
engine never waits on the exp/softmax chain. RoPE's rotate-half runs on
the vector/pool engines with partition-shifted reads (no PE perm matmul).
"""
import sys

sys.path.insert(0, "/opt/trn_rl_repo")

import numpy as np
import ml_dtypes

import concourse.bacc as bacc
import concourse.mybir as mybir
import concourse.tile as tile
from concourse.bass_utils import run_bass_kernel_spmd

N_CORES = 8
C = 2048
H = 16
D = 128
HPC = H // N_CORES          # heads per core = 2
PB = 512                    # row panel width
JB = 128                    # key tile width
CC = C // 128               # contraction chunks = 16
FT = 3 * HPC                # qkv f-tiles per core (q0 q1 k0 k1 v0 v1)
NOC = C // 128              # out-proj column tiles
NEG = -1.0e30
ROPE_BASE = 10000.0

BF = mybir.dt.bfloat16
F32 = mybir.dt.float32


def build_module(B, T):
    BT = B * T
    NPB = T // PB            # panels per batch
    NP = BT // PB            # total panels
    scale = 1.0 / float(np.sqrt(D))

    nc = bacc.Bacc("TRN2", target_bir_lowering=False, debug=False,
                   num_devices=N_CORES)

    # x pre-tiled on host: xtiles[g, p, cc*PB + r] = x[g*PB + r, cc*128 + p]
    xtiles = nc.dram_tensor("xtiles", [NP, 128, CC * PB], BF,
                            kind="ExternalInput").ap()
    wqkvT = nc.dram_tensor("wqkvT", [C, FT * 128], BF, kind="ExternalInput").ap()
    wprojT = nc.dram_tensor("wprojT", [HPC * 128, C], BF, kind="ExternalInput").ap()
    cosT = nc.dram_tensor("cosT", [128, T], BF, kind="ExternalInput").ap()
    # sinT is sign-folded on host: rows 0:64 hold -sin, rows 64:128 hold +sin
    sinT = nc.dram_tensor("sinT", [128, T], F32, kind="ExternalInput").ap()
    maskT = nc.dram_tensor("maskT", [128, 896], F32, kind="ExternalInput").ap()
    zout = nc.dram_tensor("zout", [C, BT], BF, kind="ExternalOutput").ap()

    with tile.TileContext(nc) as tc:
        with tc.tile_pool(name="sb", bufs=1) as sb, \
             tc.tile_pool(name="ps", bufs=1, space="PSUM") as ps:
            # ---- resident constants (issued on vector/scalar queues so the
            # sync/gpsimd queues start streaming x immediately) ----
            wqkv_sb = sb.tile([128, CC, FT * 128], BF, tag="wqkv", bufs=1)
            wq_src = wqkvT.rearrange("(cc p) f -> p cc f", p=128)
            HCC = CC // 2
            nc.scalar.dma_start(out=wqkv_sb[:, :HCC, :], in_=wq_src[:, :HCC, :])
            nc.sync.dma_start(out=wqkv_sb[:, HCC:, :], in_=wq_src[:, HCC:, :])
            wproj_sb = sb.tile([128, HPC, C], BF, tag="wproj", bufs=1)
            nc.scalar.dma_start(
                out=wproj_sb[:],
                in_=wprojT.rearrange("(hh p) o -> p hh o", p=128))
            cos_sb = sb.tile([128, T], BF, tag="cos", bufs=1)
            nc.scalar.dma_start(out=cos_sb[:], in_=cosT)
            sin_sb = sb.tile([128, T], F32, tag="sin", bufs=1)
            nc.scalar.dma_start(out=sin_sb[:], in_=sinT)
            mask_sb = sb.tile([128, 896], F32, tag="mask", bufs=1)
            nc.scalar.dma_start(out=mask_sb[:], in_=maskT)
            ones_col = sb.tile([128, 1], BF, tag="ones_c", bufs=1)
            nc.vector.memset(ones_col[:], 1.0)
            ones_row = sb.tile([1, 128], BF, tag="ones_r", bufs=1)
            nc.vector.memset(ones_row[:], 1.0)

            state = {}           # per-batch q/k/v tiles
            xt_q = []            # prefetched x panel tiles

            def load_xt(g):
                xt = sb.tile([128, CC, PB], BF, tag="xt", bufs=3,
                             name=f"xt_{g}")
                src = xtiles[g].rearrange("p (cc r) -> p cc r", r=PB)
                nc.sync.dma_start(out=xt[:, :HCC, :], in_=src[:, :HCC, :])
                nc.gpsimd.dma_start(out=xt[:, HCC:, :], in_=src[:, HCC:, :])
                xt_q.append(xt)

            def proj_chunks(g):
                b, pp = divmod(g, NPB)
                ts = slice(pp * PB, pp * PB + PB)
                if pp == 0:
                    state[b] = dict(
                        q=[sb.tile([128, T], BF, tag=f"q{h}", bufs=2,
                                   name=f"q{h}_{b}") for h in range(HPC)],
                        k=[sb.tile([128, T], BF, tag=f"k{h}", bufs=2,
                                   name=f"k{h}_{b}") for h in range(HPC)],
                        v=[sb.tile([128, T // 128, 128], BF, tag=f"v{h}",
                                   bufs=2, name=f"v{h}_{b}")
                           for h in range(HPC)],
                    )
                st = state[b]
                xt = xt_q.pop(0)
                for ft in range(FT):
                    pps = ps.tile([128, PB], F32, tag="mm", bufs=5)
                    for cc in range(CC):
                        nc.tensor.matmul(
                            pps[:],
                            lhsT=wqkv_sb[:, cc, ft * 128:(ft + 1) * 128],
                            rhs=xt[:, cc, :],
                            start=(cc == 0), stop=(cc == CC - 1))
                    if ft < 2 * HPC:   # q or k: apply rope
                        raw = sb.tile([128, PB], BF, tag="qkraw", bufs=3)
                        nc.scalar.copy(out=raw[:], in_=pps[:])
                        t1 = sb.tile([128, PB], F32, tag="t1", bufs=2)
                        nc.vector.tensor_mul(out=t1[:], in0=raw[:],
                                             in1=cos_sb[:, ts])
                        t2 = sb.tile([128, PB], F32, tag="t2", bufs=2)
                        # rotate-half via partition-shifted writes; sin table
                        # is pre-swapped/sign-folded on host so both SBUF
                        # inputs share a base partition (walrus requires it)
                        nc.gpsimd.tensor_mul(out=t2[0:64, :],
                                             in0=raw[64:128, :],
                                             in1=sin_sb[64:128, ts])
                        nc.gpsimd.tensor_mul(out=t2[64:128, :],
                                             in0=raw[0:64, :],
                                             in1=sin_sb[0:64, ts])
                        dest = (st["q"] if ft < HPC else st["k"])[ft % HPC]
                        nc.vector.tensor_add(out=dest[:, ts], in0=t1[:],
                                             in1=t2[:])
                    else:              # v: stage + one blocked dma-transpose
                        h = ft - 2 * HPC
                        vst = sb.tile([128, PB], BF, tag="vstage", bufs=2)
                        nc.scalar.copy(out=vst[:], in_=pps[:])
                        nc.sync.dma_start_transpose(
                            out=st["v"][h][:, pp * (PB // 128):
                                           (pp + 1) * (PB // 128), :],
                            in_=vst[:])
                    yield

            def att_chunks(g):
                b, pp = divmod(g, NPB)
                st = state[b]
                nj = (pp + 1) * (PB // JB)
                q0 = pp * PB
                den = ps.tile([64, PB], F32, tag="den", bufs=1,
                              name=f"den_{g}")
                ytil = [ps.tile([128, PB], F32, tag="ytil", bufs=2,
                                name=f"ytil{h}_{g}") for h in range(HPC)]

                def emit_S(h, j):
                    kk = j - pp * (PB // JB)
                    lo = max(kk, 0) * 128
                    sps = ps.tile([128, PB], F32, tag="mm", bufs=5,
                                  name=f"s{h}_{g}_{j}")
                    nc.tensor.matmul(
                        sps[:, lo:PB],
                        lhsT=st["k"][h][:, j * JB:(j + 1) * JB],
                        rhs=st["q"][h][:, q0 + lo:q0 + PB],
                        start=True, stop=True)
                    return sps

                def emit_fin(h, j, sps):
                    kk = j - pp * (PB // JB)
                    lo = max(kk, 0) * 128
                    e = sb.tile([128, PB], BF, tag="e", bufs=4,
                                name=f"e{h}_{g}_{j}")
                    if kk >= 0:
                        nc.vector.scalar_tensor_tensor(
                            out=sps[:, lo:lo + 128],
                            in0=sps[:, lo:lo + 128], scalar=scale,
                            in1=mask_sb[:, 384:512],
                            op0=mybir.AluOpType.mult,
                            op1=mybir.AluOpType.add)
                        nc.scalar.activation(
                            out=e[:, lo:lo + 128], in_=sps[:, lo:lo + 128],
                            func=mybir.ActivationFunctionType.Exp)
                        if lo + 128 < PB:
                            nc.scalar.activation(
                                out=e[:, lo + 128:PB],
                                in_=sps[:, lo + 128:PB],
                                func=mybir.ActivationFunctionType.Exp,
                                scale=scale)
                    else:
                        nc.scalar.activation(
                            out=e[:, lo:PB], in_=sps[:, lo:PB],
                            func=mybir.ActivationFunctionType.Exp,
                            scale=scale)
                    nc.tensor.matmul(den[32 * h:32 * h + 1, lo:PB],
                                     lhsT=ones_col[:],
                                     rhs=e[:, lo:PB], start=(j == 0),
                                     stop=(j == nj - 1))
                    nc.tensor.matmul(ytil[h][:, lo:PB],
                                     lhsT=st["v"][h][:, j, :],
                                     rhs=e[:, lo:PB], start=(j == 0),
                                     stop=(j == nj - 1))

                jobs = [(h, j) for j in range(nj) for h in range(HPC)]
                spss = {jobs[0]: emit_S(*jobs[0]),
                        jobs[1]: emit_S(*jobs[1])}
                for idx, (h, j) in enumerate(jobs):
                    if idx + 2 < len(jobs):
                        spss[jobs[idx + 2]] = emit_S(*jobs[idx + 2])
                    emit_fin(h, j, spss.pop((h, j)))
                    if idx % 2 == 1:
                        yield

                yp = sb.tile([128, HPC, PB], BF, tag="yp", bufs=2,
                             name=f"yp_{g}")
                yps[g] = yp
                for h in range(HPC):
                    dbf = sb.tile([1, PB], BF, tag="dbf", bufs=2)
                    nc.scalar.copy(out=dbf[:], in_=den[32 * h:32 * h + 1, :])
                    bc = ps.tile([128, PB], F32, tag="mm", bufs=5,
                                 name=f"bc{h}_{g}")
                    nc.tensor.matmul(bc[:], lhsT=ones_row[:],
                                     rhs=dbf[:], start=True, stop=True)
                    rec = sb.tile([128, PB], F32, tag="rec", bufs=2)
                    nc.vector.reciprocal_approx_fast(out=rec[:], in_=bc[:])
                    nc.vector.tensor_mul(out=yp[:, h, :], in0=ytil[h][:],
                                         in1=rec[:])
                    yield

            def out_chunks(g, yp):
                b, pp = divmod(g, NPB)
                r0g = b * T + pp * PB
                for ocg in range(NOC // 4):
                    zst = sb.tile([128, 4, PB], BF, tag="zst", bufs=4,
                                  name=f"zst_{g}_{ocg}")
                    for i in range(4):
                        oc = ocg * 4 + i
                        zps = ps.tile([128, PB], F32, tag="mm", bufs=5,
                                      name=f"z{g}_{oc}")
                        for hh in range(HPC):
                            nc.tensor.matmul(
                                zps[:],
                                lhsT=wproj_sb[:, hh, oc * 128:(oc + 1) * 128],
                                rhs=yp[:, hh, :],
                                start=(hh == 0), stop=(hh == HPC - 1))
                        # pool can't read PSUM; split the casts vector/scalar
                        if i % 2 == 0:
                            nc.vector.tensor_copy(out=zst[:, i, :], in_=zps[:])
                        else:
                            nc.scalar.copy(out=zst[:, i, :], in_=zps[:])
                    dst = zout[ocg * 512:(ocg + 1) * 512, r0g:r0g + PB]
                    nc.gpsimd.dma_start(
                        out=dst.rearrange("(i p) t -> p i t", p=128),
                        in_=zst[:])
                    yield

            # ---- software-pipelined emission ----
            yps = {}
            load_xt(0)
            load_xt(1)
            if NP > 2:
                load_xt(2)
            for s in range(NP + 2):
                gens = []
                if s < NP:
                    gens.append(proj_chunks(s))
                if 0 <= s - 1 < NP:
                    gens.append(att_chunks(s - 1))
                if 0 <= s - 2 < NP:
                    gens.append(out_chunks(s - 2, yps.pop(s - 2)))
                # round-robin the stage generators
                live = list(gens)
                while live:
                    nxt = []
                    for gen in live:
                        try:
                            next(gen)
                            nxt.append(gen)
                        except StopIteration:
                            pass
                    live = nxt
                if s + 3 < NP:
                    load_xt(s + 3)

    nc.compile()
    return nc


_module_cache = {}


def _get_module(B, T):
    key = (B, T)
    if key not in _module_cache:
        _module_cache[key] = build_module(B, T)
    return _module_cache[key]


def _host_prep(x, Wqkv, Wproj, B, T):
    bf16 = ml_dtypes.bfloat16
    BT = B * T
    NP = BT // PB
    x2 = x.reshape(NP, PB, CC, 128)
    xtiles = np.ascontiguousarray(
        x2.transpose(0, 3, 2, 1).reshape(NP, 128, CC * PB)).astype(bf16)

    inv = 1.0 / (ROPE_BASE ** (np.arange(0, D, 2, dtype=np.float32) / D))
    t = np.arange(T, dtype=np.float32)
    fr = np.outer(t, inv)                      # [T, 64]
    emb = np.concatenate([fr, fr], -1)         # [T, 128]
    cosT = np.ascontiguousarray(np.cos(emb).T).astype(bf16)
    sinP = np.sin(emb).T                       # [128, T]
    # swapped + sign-folded: row p<64 holds sin[p+64], row p>=64 holds -sin[p-64]
    sinT = np.ascontiguousarray(
        np.concatenate([sinP[64:128], -sinP[0:64]], 0)).astype(np.float32)

    g = np.arange(896)[None, :]
    p = np.arange(128)[:, None]
    maskT = np.where(g >= p + 384, 0.0, NEG).astype(np.float32)

    in_maps = []
    for c in range(N_CORES):
        heads = [HPC * c + h for h in range(HPC)]
        rows = []
        for blk in range(3):                   # q, k, v blocks of Wqkv
            for h in heads:
                r0 = blk * C + h * D
                rows.append(Wqkv[r0:r0 + D])
        wslice = np.concatenate(rows, 0)       # [FT*128, C]
        wqkvT = np.ascontiguousarray(wslice.T).astype(bf16)
        cols = np.concatenate([np.arange(h * D, (h + 1) * D) for h in heads])
        wprojT = np.ascontiguousarray(Wproj[:, cols].T).astype(bf16)
        in_maps.append({
            "xtiles": xtiles,
            "wqkvT": wqkvT,
            "wprojT": wprojT,
            "cosT": cosT,
            "sinT": sinT,
            "maskT": maskT,
        })
    return in_maps


last_results = None


def kernel(x, Wqkv, Wproj, _trace=False, _trace_kwargs=None):
    global last_results
    x = np.asarray(x, dtype=np.float32)
    Wqkv = np.asarray(Wqkv, dtype=np.float32)
    Wproj = np.asarray(Wproj, dtype=np.float32)
    B, T, _C = x.shape
    assert _C == C and T % PB == 0

    nc = _get_module(B, T)
    in_maps = _host_prep(x, Wqkv, Wproj, B, T)
    res = run_bass_kernel_spmd(nc, in_maps, core_ids=list(range(N_CORES)),
                               trace=_trace, **(_trace_kwargs or {}))
    last_results = res
    z = res.results[0]["zout"].astype(np.float32)
    for c in range(1, N_CORES):
        z += res.results[c]["zout"].astype(np.float32)
    y = np.ascontiguousarray(z.T).reshape(B, T, C)
    return y


# revision 14
# speedup vs baseline: 1.4136x; 1.0070x over previous
"""Causal self-attention (QKV proj + RoPE + causal SDPA + out proj) on 8 trn2 cores.

Sharding: tensor-parallel over heads. Each core owns 2 of 16 heads:
  - Wqkv column-split (the core's q/k/v head rows), Wproj row-split.
  - Each core computes a full-shape partial of the output projection;
    the 8 partials are summed (and transposed back) on the host.

Device-side layout: everything runs transposed (x^T on chip); the QKV
projection computes qkv^T = Wslice @ x with the head dim on partitions,
which is what Q@K^T and the output projection want as inputs. V is
transposed on chip with DMA xbar transposes.

Pipelined schedule: the per-panel work is split into three stages
(QKV-proj+rope for panel g, attention for panel g-1, out-proj for panel
g-2) whose instruction chunks are interleaved in emission order, so every
engine consumes operands produced at least a stage earlier and the tensor
engine never waits on the exp/softmax chain. RoPE's rotate-half runs on
the vector/pool engines with partition-shifted reads (no PE perm matmul).
"""
import sys

sys.path.insert(0, "/opt/trn_rl_repo")

import numpy as np
import ml_dtypes

import concourse.bacc as bacc
import concourse.mybir as mybir
import concourse.tile as tile
from concourse.bass_utils import run_bass_kernel_spmd

N_CORES = 8
C = 2048
H = 16
D = 128
HPC = H // N_CORES          # heads per core = 2
PB = 512                    # row panel width
JB = 128                    # key tile width
CC = C // 128               # contraction chunks = 16
FT = 3 * HPC                # qkv f-tiles per core (q0 q1 k0 k1 v0 v1)
NOC = C // 128              # out-proj column tiles
NEG = -1.0e30
ROPE_BASE = 10000.0

BF = mybir.dt.bfloat16
F32 = mybir.dt.float32


def build_module(B, T):
    BT = B * T
    NPB = T // PB            # panels per batch
    NP = BT // PB            # total panels
    HCC = CC // 2
    scale = 1.0 / float(np.sqrt(D))

    nc = bacc.Bacc("TRN2", target_bir_lowering=False, debug=False,
                   num_devices=N_CORES)

    # x pre-tiled on host: xtiles[g, p, cc*PB + r] = x[g*PB + r, cc*128 + p]
    xtiles = nc.dram_tensor("xtiles", [NP, 128, CC * PB], BF,
                            kind="ExternalInput").ap()
    wqkvT = nc.dram_tensor("wqkvT", [C, FT * 128], BF, kind="ExternalInput").ap()
    wprojT = nc.dram_tensor("wprojT", [HPC * 128, C], BF, kind="ExternalInput").ap()
    cosT = nc.dram_tensor("cosT", [128, T], BF, kind="ExternalInput").ap()
    # sinT is swapped + sign-folded on host (see _host_prep)
    sinT = nc.dram_tensor("sinT", [128, T], BF, kind="ExternalInput").ap()
    maskT = nc.dram_tensor("maskT", [128, 896], F32, kind="ExternalInput").ap()
    zout = nc.dram_tensor("zout", [C, BT], BF, kind="ExternalOutput").ap()

    with tile.TileContext(nc) as tc:
        with tc.tile_pool(name="sb", bufs=1) as sb, \
             tc.tile_pool(name="ps", bufs=1, space="PSUM") as ps:
            # ---- resident constants (issued on vector/scalar queues so the
            # sync/gpsimd queues start streaming x immediately) ----
            # weights loaded in cc-chunks across three queues so the first
            # proj matmuls can start as soon as their chunk lands
            wqkv_sb = sb.tile([128, CC, FT * 128], BF, tag="wqkv", bufs=1)
            wq_src = wqkvT.rearrange("(cc p) f -> p cc f", p=128)
            qs = [nc.scalar, nc.sync, nc.gpsimd]
            for ci in range(4):
                sl = slice(ci * 4, (ci + 1) * 4)
                qs[ci % 3].dma_start(out=wqkv_sb[:, sl, :], in_=wq_src[:, sl, :])
            wproj_sb = sb.tile([128, HPC, C], BF, tag="wproj", bufs=1)
            nc.scalar.dma_start(
                out=wproj_sb[:],
                in_=wprojT.rearrange("(hh p) o -> p hh o", p=128))
            cos_sb = sb.tile([128, T], BF, tag="cos", bufs=1)
            nc.scalar.dma_start(out=cos_sb[:], in_=cosT)
            sin_sb = sb.tile([128, T], BF, tag="sin", bufs=1)
            nc.scalar.dma_start(out=sin_sb[:], in_=sinT)
            mask_sb = sb.tile([128, 896], F32, tag="mask", bufs=1)
            nc.scalar.dma_start(out=mask_sb[:], in_=maskT)
            ones_col = sb.tile([128, 1], BF, tag="ones_c", bufs=1)
            nc.vector.memset(ones_col[:], 1.0)
            ones_row = sb.tile([1, 128], BF, tag="ones_r", bufs=1)
            nc.vector.memset(ones_row[:], 1.0)

            state = {}           # per-batch q/k/v tiles
            xt_q = []            # prefetched x panel tiles

            def load_xt(g):
                xt = sb.tile([128, CC, PB], BF, tag="xt", bufs=3,
                             name=f"xt_{g}")
                src = xtiles[g].rearrange("p (cc r) -> p cc r", r=PB)
                nc.sync.dma_start(out=xt[:, :HCC, :], in_=src[:, :HCC, :])
                nc.gpsimd.dma_start(out=xt[:, HCC:, :], in_=src[:, HCC:, :])
                xt_q.append(xt)

            def proj_chunks(g):
                b, pp = divmod(g, NPB)
                ts = slice(pp * PB, pp * PB + PB)
                if pp == 0:
                    state[b] = dict(
                        q=[sb.tile([128, T], BF, tag=f"q{h}", bufs=2,
                                   name=f"q{h}_{b}") for h in range(HPC)],
                        k=[sb.tile([128, T], BF, tag=f"k{h}", bufs=2,
                                   name=f"k{h}_{b}") for h in range(HPC)],
                        v=[sb.tile([128, T // 128, 128], BF, tag=f"v{h}",
                                   bufs=2, name=f"v{h}_{b}")
                           for h in range(HPC)],
                    )
                st = state[b]
                xt = xt_q.pop(0)
                for ft in range(FT):
                    pps = ps.tile([128, PB], F32, tag="mm", bufs=5)
                    for cc in range(CC):
                        nc.tensor.matmul(
                            pps[:],
                            lhsT=wqkv_sb[:, cc, ft * 128:(ft + 1) * 128],
                            rhs=xt[:, cc, :],
                            start=(cc == 0), stop=(cc == CC - 1))
                    if ft < 2 * HPC:   # q or k: apply rope
                        raw = sb.tile([128, PB], BF, tag="qkraw", bufs=3)
                        nc.scalar.copy(out=raw[:], in_=pps[:])
                        t1 = sb.tile([128, PB], BF, tag="t1", bufs=2)
                        nc.vector.tensor_mul(out=t1[:], in0=raw[:],
                                             in1=cos_sb[:, ts])
                        t2 = sb.tile([128, PB], BF, tag="t2", bufs=2)
                        # rotate-half via partition-shifted writes; sin table
                        # is pre-swapped/sign-folded on host so both SBUF
                        # inputs share a base partition (walrus requires it)
                        nc.gpsimd.tensor_mul(out=t2[0:64, :],
                                             in0=raw[64:128, :],
                                             in1=sin_sb[64:128, ts])
                        nc.gpsimd.tensor_mul(out=t2[64:128, :],
                                             in0=raw[0:64, :],
                                             in1=sin_sb[0:64, ts])
                        dest = (st["q"] if ft < HPC else st["k"])[ft % HPC]
                        nc.vector.tensor_add(out=dest[:, ts], in0=t1[:],
                                             in1=t2[:])
                    else:              # v: stage + one blocked dma-transpose
                        h = ft - 2 * HPC
                        vst = sb.tile([128, PB], BF, tag="vstage", bufs=2)
                        nc.scalar.copy(out=vst[:], in_=pps[:])
                        nc.sync.dma_start_transpose(
                            out=st["v"][h][:, pp * (PB // 128):
                                           (pp + 1) * (PB // 128), :],
                            in_=vst[:])
                    yield

            def att_chunks(g):
                b, pp = divmod(g, NPB)
                st = state[b]
                nj = (pp + 1) * (PB // JB)
                q0 = pp * PB
                den = ps.tile([64, PB], F32, tag="den", bufs=1,
                              name=f"den_{g}")
                ytil = [ps.tile([128, PB], F32, tag="ytil", bufs=2,
                                name=f"ytil{h}_{g}") for h in range(HPC)]

                def emit_S(h, j):
                    kk = j - pp * (PB // JB)
                    lo = max(kk, 0) * 128
                    sps = ps.tile([128, PB], F32, tag="mm", bufs=5,
                                  name=f"s{h}_{g}_{j}")
                    nc.tensor.matmul(
                        sps[:, lo:PB],
                        lhsT=st["k"][h][:, j * JB:(j + 1) * JB],
                        rhs=st["q"][h][:, q0 + lo:q0 + PB],
                        start=True, stop=True)
                    return sps

                def emit_fin(h, j, sps):
                    kk = j - pp * (PB // JB)
                    lo = max(kk, 0) * 128
                    e = sb.tile([128, PB], BF, tag="e", bufs=4,
                                name=f"e{h}_{g}_{j}")
                    if kk >= 0:
                        nc.vector.scalar_tensor_tensor(
                            out=sps[:, lo:lo + 128],
                            in0=sps[:, lo:lo + 128], scalar=scale,
                            in1=mask_sb[:, 384:512],
                            op0=mybir.AluOpType.mult,
                            op1=mybir.AluOpType.add)
                        nc.scalar.activation(
                            out=e[:, lo:lo + 128], in_=sps[:, lo:lo + 128],
                            func=mybir.ActivationFunctionType.Exp)
                        if lo + 128 < PB:
                            nc.scalar.activation(
                                out=e[:, lo + 128:PB],
                                in_=sps[:, lo + 128:PB],
                                func=mybir.ActivationFunctionType.Exp,
                                scale=scale)
                    else:
                        nc.scalar.activation(
                            out=e[:, lo:PB], in_=sps[:, lo:PB],
                            func=mybir.ActivationFunctionType.Exp,
                            scale=scale)
                    nc.tensor.matmul(den[32 * h:32 * h + 1, lo:PB],
                                     lhsT=ones_col[:],
                                     rhs=e[:, lo:PB], start=(j == 0),
                                     stop=(j == nj - 1))
                    nc.tensor.matmul(ytil[h][:, lo:PB],
                                     lhsT=st["v"][h][:, j, :],
                                     rhs=e[:, lo:PB], start=(j == 0),
                                     stop=(j == nj - 1))

                jobs = [(h, j) for j in range(nj) for h in range(HPC)]
                spss = {jobs[0]: emit_S(*jobs[0]),
                        jobs[1]: emit_S(*jobs[1])}
                for idx, (h, j) in enumerate(jobs):
                    if idx + 2 < len(jobs):
                        spss[jobs[idx + 2]] = emit_S(*jobs[idx + 2])
                    emit_fin(h, j, spss.pop((h, j)))
                    if idx % 2 == 1:
                        yield

                yp = sb.tile([128, HPC, PB], BF, tag="yp", bufs=2,
                             name=f"yp_{g}")
                yps[g] = yp
                for h in range(HPC):
                    dbf = sb.tile([1, PB], BF, tag="dbf", bufs=2)
                    nc.scalar.copy(out=dbf[:], in_=den[32 * h:32 * h + 1, :])
                    bc = ps.tile([128, PB], F32, tag="mm", bufs=5,
                                 name=f"bc{h}_{g}")
                    nc.tensor.matmul(bc[:], lhsT=ones_row[:],
                                     rhs=dbf[:], start=True, stop=True)
                    rec = sb.tile([128, PB], F32, tag="rec", bufs=2)
                    nc.vector.reciprocal_approx_fast(out=rec[:], in_=bc[:])
                    nc.vector.tensor_mul(out=yp[:, h, :], in0=ytil[h][:],
                                         in1=rec[:])
                    yield

            def out_chunks(g, yp):
                b, pp = divmod(g, NPB)
                r0g = b * T + pp * PB
                for ocg in range(NOC // 4):
                    zst = sb.tile([128, 4, PB], BF, tag="zst", bufs=4,
                                  name=f"zst_{g}_{ocg}")
                    for i in range(4):
                        oc = ocg * 4 + i
                        zps = ps.tile([128, PB], F32, tag="mm", bufs=5,
                                      name=f"z{g}_{oc}")
                        for hh in range(HPC):
                            nc.tensor.matmul(
                                zps[:],
                                lhsT=wproj_sb[:, hh, oc * 128:(oc + 1) * 128],
                                rhs=yp[:, hh, :],
                                start=(hh == 0), stop=(hh == HPC - 1))
                        # pool can't read PSUM; split the casts vector/scalar
                        if i % 2 == 0:
                            nc.vector.tensor_copy(out=zst[:, i, :], in_=zps[:])
                        else:
                            nc.scalar.copy(out=zst[:, i, :], in_=zps[:])
                    dst = zout[ocg * 512:(ocg + 1) * 512, r0g:r0g + PB]
                    # rotate store queues so xt loads never sit behind stores
                    qeng = (nc.gpsimd, nc.sync, nc.scalar)[(g + ocg) % 3]
                    qeng.dma_start(
                        out=dst.rearrange("(i p) t -> p i t", p=128),
                        in_=zst[:])
                    yield

            # ---- software-pipelined emission ----
            yps = {}
            load_xt(0)
            load_xt(1)
            if NP > 2:
                load_xt(2)
            for s in range(NP + 2):
                gens = []
                if s < NP:
                    gens.append(proj_chunks(s))
                if 0 <= s - 1 < NP:
                    gens.append(att_chunks(s - 1))
                if 0 <= s - 2 < NP:
                    gens.append(out_chunks(s - 2, yps.pop(s - 2)))
                # round-robin the stage generators
                live = list(gens)
                while live:
                    nxt = []
                    for gen in live:
                        try:
                            next(gen)
                            nxt.append(gen)
                        except StopIteration:
                            pass
                    live = nxt
                if s + 3 < NP:
                    load_xt(s + 3)

    nc.compile()
    return nc


_module_cache = {}


def _get_module(B, T):
    key = (B, T)
    if key not in _module_cache:
        _module_cache[key] = build_module(B, T)
    return _module_cache[key]


def _host_prep(x, Wqkv, Wproj, B, T):
    bf16 = ml_dtypes.bfloat16
    BT = B * T
    NP = BT // PB
    x2 = x.reshape(NP, PB, CC, 128)
    xtiles = np.ascontiguousarray(
        x2.transpose(0, 3, 2, 1).reshape(NP, 128, CC * PB)).astype(bf16)

    inv = 1.0 / (ROPE_BASE ** (np.arange(0, D, 2, dtype=np.float32) / D))
    t = np.arange(T, dtype=np.float32)
    fr = np.outer(t, inv)                      # [T, 64]
    emb = np.concatenate([fr, fr], -1)         # [T, 128]
    cosT = np.ascontiguousarray(np.cos(emb).T).astype(bf16)
    sinP = np.sin(emb).T                       # [128, T]
    # swapped + sign-folded: row p<64 holds sin[p+64], row p>=64 holds -sin[p-64]
    sinT = np.ascontiguousarray(
        np.concatenate([sinP[64:128], -sinP[0:64]], 0)).astype(bf16)

    g = np.arange(896)[None, :]
    p = np.arange(128)[:, None]
    maskT = np.where(g >= p + 384, 0.0, NEG).astype(np.float32)

    in_maps = []
    for c in range(N_CORES):
        heads = [HPC * c + h for h in range(HPC)]
        rows = []
        for blk in range(3):                   # q, k, v blocks of Wqkv
            for h in heads:
                r0 = blk * C + h * D
                rows.append(Wqkv[r0:r0 + D])
        wslice = np.concatenate(rows, 0)       # [FT*128, C]
        wqkvT = np.ascontiguousarray(wslice.T).astype(bf16)
        cols = np.concatenate([np.arange(h * D, (h + 1) * D) for h in heads])
        wprojT = np.ascontiguousarray(Wproj[:, cols].T).astype(bf16)
        in_maps.append({
            "xtiles": xtiles,
            "wqkvT": wqkvT,
            "wprojT": wprojT,
            "cosT": cosT,
            "sinT": sinT,
            "maskT": maskT,
        })
    return in_maps


last_results = None


def kernel(x, Wqkv, Wproj, _trace=False, _trace_kwargs=None):
    global last_results
    x = np.asarray(x, dtype=np.float32)
    Wqkv = np.asarray(Wqkv, dtype=np.float32)
    Wproj = np.asarray(Wproj, dtype=np.float32)
    B, T, _C = x.shape
    assert _C == C and T % PB == 0

    nc = _get_module(B, T)
    in_maps = _host_prep(x, Wqkv, Wproj, B, T)
    res = run_bass_kernel_spmd(nc, in_maps, core_ids=list(range(N_CORES)),
                               trace=_trace, **(_trace_kwargs or {}))
    last_results = res
    z = res.results[0]["zout"].astype(np.float32)
    for c in range(1, N_CORES):
        z += res.results[c]["zout"].astype(np.float32)
    y = np.ascontiguousarray(z.T).reshape(B, T, C)
    return y


# revision 20
# speedup vs baseline: 1.4393x; 1.0182x over previous
"""Causal self-attention (QKV proj + RoPE + causal SDPA + out proj) on 8 trn2 cores.

Sharding: tensor-parallel over heads. Each core owns 2 of 16 heads:
  - Wqkv column-split (the core's q/k/v head rows), Wproj row-split.
  - Each core computes a full-shape partial of the output projection;
    the 8 partials are summed (and transposed back) on the host.

Device-side layout: everything runs transposed (x^T on chip); the QKV
projection computes qkv^T = Wslice @ x with the head dim on partitions,
which is what Q@K^T and the output projection want as inputs. V is
transposed on chip with DMA xbar transposes.

Pipelined schedule: the per-panel work is split into three stages
(QKV-proj+rope for panel g, attention for panel g-1, out-proj for panel
g-2) whose instruction chunks are interleaved in emission order, so every
engine consumes operands produced at least a stage earlier and the tensor
engine never waits on the exp/softmax chain. RoPE's rotate-half runs on
the vector/pool engines with partition-shifted reads (no PE perm matmul).
"""
import sys

sys.path.insert(0, "/opt/trn_rl_repo")

import numpy as np
import ml_dtypes

import concourse.bacc as bacc
import concourse.mybir as mybir
import concourse.tile as tile
from concourse.bass_utils import run_bass_kernel_spmd

N_CORES = 8
C = 2048
H = 16
D = 128
HPC = H // N_CORES          # heads per core = 2
PB = 512                    # row panel width
JB = 128                    # key tile width
CC = C // 128               # contraction chunks = 16
FT = 3 * HPC                # qkv f-tiles per core (q0 q1 k0 k1 v0 v1)
NOC = C // 128              # out-proj column tiles
NEG = -1.0e30
ROPE_BASE = 10000.0

BF = mybir.dt.bfloat16
F32 = mybir.dt.float32


def build_module(B, T):
    BT = B * T
    NPB = T // PB            # panels per batch
    NP = BT // PB            # total panels
    HCC = CC // 2
    scale = 1.0 / float(np.sqrt(D))

    nc = bacc.Bacc("TRN2", target_bir_lowering=False, debug=False,
                   num_devices=N_CORES)

    # x pre-tiled on host: xtiles[g, p, cc*PB + r] = x[g*PB + r, cc*128 + p]
    xtiles = nc.dram_tensor("xtiles", [NP, 128, CC * PB], BF,
                            kind="ExternalInput").ap()
    wqkvT = nc.dram_tensor("wqkvT", [C, FT * 128], BF, kind="ExternalInput").ap()
    wprojT = nc.dram_tensor("wprojT", [HPC * 128, C], BF, kind="ExternalInput").ap()
    cosT = nc.dram_tensor("cosT", [128, T], BF, kind="ExternalInput").ap()
    # sinT is swapped + sign-folded on host (see _host_prep)
    sinT = nc.dram_tensor("sinT", [128, T], BF, kind="ExternalInput").ap()
    maskT = nc.dram_tensor("maskT", [128, 896], F32, kind="ExternalInput").ap()
    zout = nc.dram_tensor("zout", [C, BT], BF, kind="ExternalOutput").ap()

    with tile.TileContext(nc) as tc:
        with tc.tile_pool(name="sb", bufs=1) as sb, \
             tc.tile_pool(name="ps", bufs=1, space="PSUM") as ps:
            # ---- resident constants (issued on vector/scalar queues so the
            # sync/gpsimd queues start streaming x immediately) ----
            # weights loaded in cc-chunks across three queues so the first
            # proj matmuls can start as soon as their chunk lands
            wqkv_sb = sb.tile([128, CC, FT * 128], BF, tag="wqkv", bufs=1)
            wq_src = wqkvT.rearrange("(cc p) f -> p cc f", p=128)
            qs = [nc.scalar, nc.sync, nc.gpsimd]
            for ci in range(4):
                sl = slice(ci * 4, (ci + 1) * 4)
                qs[ci % 3].dma_start(out=wqkv_sb[:, sl, :], in_=wq_src[:, sl, :])
            wproj_sb = sb.tile([128, HPC, C], BF, tag="wproj", bufs=1)
            nc.scalar.dma_start(
                out=wproj_sb[:],
                in_=wprojT.rearrange("(hh p) o -> p hh o", p=128))
            cos_sb = sb.tile([128, T], BF, tag="cos", bufs=1)
            nc.scalar.dma_start(out=cos_sb[:], in_=cosT)
            sin_sb = sb.tile([128, T], BF, tag="sin", bufs=1)
            nc.scalar.dma_start(out=sin_sb[:], in_=sinT)
            mask_sb = sb.tile([128, 896], F32, tag="mask", bufs=1)
            nc.scalar.dma_start(out=mask_sb[:], in_=maskT)
            # all-ones square: the denominator matmul broadcasts sum(e) to all
            # 128 psum rows, so no separate bc/broadcast matmul is needed
            ones_sq = sb.tile([128, 128], BF, tag="ones_s", bufs=1)
            nc.vector.memset(ones_sq[:], 1.0)

            state = {}           # per-batch q/k/v tiles
            xt_q = []            # prefetched x panel tiles

            def load_xt(g):
                xt = sb.tile([128, CC, PB], BF, tag="xt", bufs=3,
                             name=f"xt_{g}")
                src = xtiles[g].rearrange("p (cc r) -> p cc r", r=PB)
                nc.sync.dma_start(out=xt[:, :HCC, :], in_=src[:, :HCC, :])
                nc.gpsimd.dma_start(out=xt[:, HCC:, :], in_=src[:, HCC:, :])
                xt_q.append(xt)

            def proj_chunks(g):
                b, pp = divmod(g, NPB)
                ts = slice(pp * PB, pp * PB + PB)
                if pp == 0:
                    state[b] = dict(
                        q=[sb.tile([128, T], BF, tag=f"q{h}", bufs=2,
                                   name=f"q{h}_{b}") for h in range(HPC)],
                        k=[sb.tile([128, T], BF, tag=f"k{h}", bufs=2,
                                   name=f"k{h}_{b}") for h in range(HPC)],
                        v=[sb.tile([128, T // 128, 128], BF, tag=f"v{h}",
                                   bufs=2, name=f"v{h}_{b}")
                           for h in range(HPC)],
                    )
                st = state[b]
                xt = xt_q.pop(0)
                for ft in range(FT):
                    pps = ps.tile([128, PB], F32, tag="mm", bufs=4)
                    for cc in range(CC):
                        nc.tensor.matmul(
                            pps[:],
                            lhsT=wqkv_sb[:, cc, ft * 128:(ft + 1) * 128],
                            rhs=xt[:, cc, :],
                            start=(cc == 0), stop=(cc == CC - 1))
                    if ft < 2 * HPC:   # q or k: apply rope
                        raw = sb.tile([128, PB], BF, tag="qkraw", bufs=3)
                        nc.scalar.copy(out=raw[:], in_=pps[:])
                        t1 = sb.tile([128, PB], BF, tag="t1", bufs=2)
                        nc.vector.tensor_mul(out=t1[:], in0=raw[:],
                                             in1=cos_sb[:, ts])
                        t2 = sb.tile([128, PB], BF, tag="t2", bufs=2)
                        # rotate-half via partition-shifted writes; sin table
                        # is pre-swapped/sign-folded on host so both SBUF
                        # inputs share a base partition (walrus requires it)
                        nc.gpsimd.tensor_mul(out=t2[0:64, :],
                                             in0=raw[64:128, :],
                                             in1=sin_sb[64:128, ts])
                        nc.gpsimd.tensor_mul(out=t2[64:128, :],
                                             in0=raw[0:64, :],
                                             in1=sin_sb[0:64, ts])
                        dest = (st["q"] if ft < HPC else st["k"])[ft % HPC]
                        nc.vector.tensor_add(out=dest[:, ts], in0=t1[:],
                                             in1=t2[:])
                    else:              # v: stage + one blocked dma-transpose
                        h = ft - 2 * HPC
                        vst = sb.tile([128, PB], BF, tag="vstage", bufs=2)
                        nc.scalar.copy(out=vst[:], in_=pps[:])
                        nc.sync.dma_start_transpose(
                            out=st["v"][h][:, pp * (PB // 128):
                                           (pp + 1) * (PB // 128), :],
                            in_=vst[:])
                    yield

            def att_chunks(g):
                b, pp = divmod(g, NPB)
                st = state[b]
                nj = (pp + 1) * (PB // JB)
                q0 = pp * PB
                den = [ps.tile([128, PB], F32, tag="den", bufs=2,
                               name=f"den{h}_{g}") for h in range(HPC)]
                ytil = [ps.tile([128, PB], F32, tag="ytil", bufs=2,
                                name=f"ytil{h}_{g}") for h in range(HPC)]

                def emit_S(h, j):
                    kk = j - pp * (PB // JB)
                    lo = max(kk, 0) * 128
                    sps = ps.tile([128, PB], F32, tag="mm", bufs=4,
                                  name=f"s{h}_{g}_{j}")
                    nc.tensor.matmul(
                        sps[:, lo:PB],
                        lhsT=st["k"][h][:, j * JB:(j + 1) * JB],
                        rhs=st["q"][h][:, q0 + lo:q0 + PB],
                        start=True, stop=True)
                    return sps

                def emit_fin(h, j, sps):
                    kk = j - pp * (PB // JB)
                    lo = max(kk, 0) * 128
                    e = sb.tile([128, PB], BF, tag="e", bufs=6,
                                name=f"e{h}_{g}_{j}")
                    if kk >= 0:
                        nc.vector.scalar_tensor_tensor(
                            out=sps[:, lo:lo + 128],
                            in0=sps[:, lo:lo + 128], scalar=scale,
                            in1=mask_sb[:, 384:512],
                            op0=mybir.AluOpType.mult,
                            op1=mybir.AluOpType.add)
                        nc.scalar.activation(
                            out=e[:, lo:lo + 128], in_=sps[:, lo:lo + 128],
                            func=mybir.ActivationFunctionType.Exp)
                        if lo + 128 < PB:
                            nc.scalar.activation(
                                out=e[:, lo + 128:PB],
                                in_=sps[:, lo + 128:PB],
                                func=mybir.ActivationFunctionType.Exp,
                                scale=scale)
                    else:
                        nc.scalar.activation(
                            out=e[:, lo:PB], in_=sps[:, lo:PB],
                            func=mybir.ActivationFunctionType.Exp,
                            scale=scale)
                    nc.tensor.matmul(den[h][:, lo:PB],
                                     lhsT=ones_sq[:],
                                     rhs=e[:, lo:PB], start=(j == 0),
                                     stop=(j == nj - 1))
                    nc.tensor.matmul(ytil[h][:, lo:PB],
                                     lhsT=st["v"][h][:, j, :],
                                     rhs=e[:, lo:PB], start=(j == 0),
                                     stop=(j == nj - 1))

                jobs = [(h, j) for j in range(nj) for h in range(HPC)]
                spss = {jobs[0]: emit_S(*jobs[0]),
                        jobs[1]: emit_S(*jobs[1])}
                for idx, (h, j) in enumerate(jobs):
                    if idx + 2 < len(jobs):
                        spss[jobs[idx + 2]] = emit_S(*jobs[idx + 2])
                    emit_fin(h, j, spss.pop((h, j)))
                    if idx % 2 == 1:
                        yield

                yp = sb.tile([128, HPC, PB], BF, tag="yp", bufs=2,
                             name=f"yp_{g}")
                yps[g] = yp
                for h in range(HPC):
                    rec = sb.tile([128, PB], F32, tag="rec", bufs=2)
                    nc.vector.reciprocal_approx_fast(out=rec[:], in_=den[h][:])
                    nc.vector.tensor_mul(out=yp[:, h, :], in0=ytil[h][:],
                                         in1=rec[:])
                    yield

            def out_chunks(g, yp):
                b, pp = divmod(g, NPB)
                r0g = b * T + pp * PB
                for ocg in range(NOC // 4):
                    zst = sb.tile([128, 4, PB], BF, tag="zst", bufs=4,
                                  name=f"zst_{g}_{ocg}")
                    for i in range(4):
                        oc = ocg * 4 + i
                        zps = ps.tile([128, PB], F32, tag="mm", bufs=4,
                                      name=f"z{g}_{oc}")
                        for hh in range(HPC):
                            nc.tensor.matmul(
                                zps[:],
                                lhsT=wproj_sb[:, hh, oc * 128:(oc + 1) * 128],
                                rhs=yp[:, hh, :],
                                start=(hh == 0), stop=(hh == HPC - 1))
                        # pool can't read PSUM; split the casts vector/scalar
                        if i % 2 == 0:
                            nc.vector.tensor_copy(out=zst[:, i, :], in_=zps[:])
                        else:
                            nc.scalar.copy(out=zst[:, i, :], in_=zps[:])
                    # rotate store queues (and split each store in half) so
                    # xt loads never sit behind stores and the tail drains fast
                    rot = (nc.gpsimd, nc.sync, nc.scalar)
                    for half in range(2):
                        r0 = ocg * 512 + half * 256
                        dst = zout[r0:r0 + 256, r0g:r0g + PB]
                        rot[(g + 2 * ocg + half) % 3].dma_start(
                            out=dst.rearrange("(i p) t -> p i t", p=128),
                            in_=zst[:, 2 * half:2 * half + 2, :])
                    yield

            # ---- software-pipelined emission ----
            yps = {}
            load_xt(0)
            load_xt(1)
            if NP > 2:
                load_xt(2)
            for s in range(NP + 2):
                gens = []
                if s < NP:
                    gens.append(proj_chunks(s))
                if 0 <= s - 1 < NP:
                    gens.append(att_chunks(s - 1))
                if 0 <= s - 2 < NP:
                    gens.append(out_chunks(s - 2, yps.pop(s - 2)))
                # round-robin the stage generators
                live = list(gens)
                while live:
                    nxt = []
                    for gen in live:
                        try:
                            next(gen)
                            nxt.append(gen)
                        except StopIteration:
                            pass
                    live = nxt
                if s + 3 < NP:
                    load_xt(s + 3)

    nc.compile()
    return nc


_module_cache = {}


def _get_module(B, T):
    key = (B, T)
    if key not in _module_cache:
        _module_cache[key] = build_module(B, T)
    return _module_cache[key]


def _host_prep(x, Wqkv, Wproj, B, T):
    bf16 = ml_dtypes.bfloat16
    BT = B * T
    NP = BT // PB
    x2 = x.reshape(NP, PB, CC, 128)
    xtiles = np.ascontiguousarray(
        x2.transpose(0, 3, 2, 1).reshape(NP, 128, CC * PB)).astype(bf16)

    inv = 1.0 / (ROPE_BASE ** (np.arange(0, D, 2, dtype=np.float32) / D))
    t = np.arange(T, dtype=np.float32)
    fr = np.outer(t, inv)                      # [T, 64]
    emb = np.concatenate([fr, fr], -1)         # [T, 128]
    cosT = np.ascontiguousarray(np.cos(emb).T).astype(bf16)
    sinP = np.sin(emb).T                       # [128, T]
    # swapped + sign-folded: row p<64 holds sin[p+64], row p>=64 holds -sin[p-64]
    sinT = np.ascontiguousarray(
        np.concatenate([sinP[64:128], -sinP[0:64]], 0)).astype(bf16)

    g = np.arange(896)[None, :]
    p = np.arange(128)[:, None]
    maskT = np.where(g >= p + 384, 0.0, NEG).astype(np.float32)

    in_maps = []
    for c in range(N_CORES):
        heads = [HPC * c + h for h in range(HPC)]
        rows = []
        for blk in range(3):                   # q, k, v blocks of Wqkv
            for h in heads:
                r0 = blk * C + h * D
                rows.append(Wqkv[r0:r0 + D])
        wslice = np.concatenate(rows, 0)       # [FT*128, C]
        wqkvT = np.ascontiguousarray(wslice.T).astype(bf16)
        cols = np.concatenate([np.arange(h * D, (h + 1) * D) for h in heads])
        wprojT = np.ascontiguousarray(Wproj[:, cols].T).astype(bf16)
        in_maps.append({
            "xtiles": xtiles,
            "wqkvT": wqkvT,
            "wprojT": wprojT,
            "cosT": cosT,
            "sinT": sinT,
            "maskT": maskT,
        })
    return in_maps


last_results = None


def kernel(x, Wqkv, Wproj, _trace=False, _trace_kwargs=None):
    global last_results
    x = np.asarray(x, dtype=np.float32)
    Wqkv = np.asarray(Wqkv, dtype=np.float32)
    Wproj = np.asarray(Wproj, dtype=np.float32)
    B, T, _C = x.shape
    assert _C == C and T % PB == 0

    nc = _get_module(B, T)
    in_maps = _host_prep(x, Wqkv, Wproj, B, T)
    res = run_bass_kernel_spmd(nc, in_maps, core_ids=list(range(N_CORES)),
                               trace=_trace, **(_trace_kwargs or {}))
    last_results = res
    z = res.results[0]["zout"].astype(np.float32)
    for c in range(1, N_CORES):
        z += res.results[c]["zout"].astype(np.float32)
    y = np.ascontiguousarray(z.T).reshape(B, T, C)
    return y
